# revision 13
# baseline (speedup 1.0000x reference)
"""Trainium2 Bass kernel for nn_Decoder (MusicVAE-style hierarchical decoder).

fp8 DoubleRow version. Strategy (8 NeuronCores, data-parallel over batch):
  - All matmuls in fp8 e4m3 with DoubleRow perf mode (2 k-tiles of 128 per
    instruction), weights pre-scaled by 16 (descale folded into the
    activation `scale=1/16`), PSUM accumulation in fp32.
  - Biases are injected into PSUM via DoubleRow matmuls whose moving operand
    is nonzero only on partition 0 (hi/lo fp8 pair for near-exact bias).
  - Conductor (16 sequential levels, batch 32/core): gates recomputed from
    z each level (no gz precompute); per-level ge table (= dec_Wih[:, :H]
    @ emb + dec_b, scaled by 16) is computed during the conductor and
    stored as an fp8 hi+lo pair.
  - Decoder: 16 levels batched (512 rows/core), 16 note steps. Gate psum =
    ge-inject (identity stationary, hi/lo pair) + Whh·h (4 DR) + Wn·note
    (2 DR). Output projection interleaved so the note->next-gates latency
    hides under gate matmuls of the next step.
  - h and note kept in fp8; c state fp32; activations bf16 out.
"""
import numpy as np
import ml_dtypes

import concourse.bacc as bacc
import concourse.tile as tile
import concourse.mybir as mybir
from concourse.bass_utils import run_bass_kernel_spmd

f8 = ml_dtypes.float8_e4m3
bf16 = ml_dtypes.bfloat16
F32 = mybir.dt.float32
BF = mybir.dt.bfloat16
F8 = mybir.dt.float8e4
AF = mybir.ActivationFunctionType
DR = mybir.MatmulPerfMode.DoubleRow

NCORES = 8
B, Z, H, T = 256, 512, 1024, 512
L, NS = 16, 16
Bc = B // NCORES            # 32 batch rows per core
R = L * Bc                  # 512 decoder rows per core
HK, TK, ZK = H // 128, T // 128, Z // 128   # 8, 4, 4
G = 4 * H // 128            # 32 gate chunks of 128
SCALE = 16.0
INV = 1.0 / SCALE


def _declare(nc):
    d = {}
    ei = dict(kind="ExternalInput")
    d["zT"] = nc.dram_tensor("zT", [128, ZK, R], F8, **ei)
    d["h0T"] = nc.dram_tensor("h0T", [128, HK, R], F8, **ei)
    d["c0T"] = nc.dram_tensor("c0T", [128, HK, R], F32, **ei)
    d["cwih"] = nc.dram_tensor("cwih", [128, ZK, 4 * H], F8, **ei)
    d["cwhh"] = nc.dram_tensor("cwhh", [128, HK, 4 * H], F8, **ei)
    d["cbias"] = nc.dram_tensor("cbias", [128, 2, 4 * H], F8, **ei)
    d["dwe"] = nc.dram_tensor("dwe", [128, HK, 4 * H], F8, **ei)
    d["dwhh"] = nc.dram_tensor("dwhh", [128, HK, 4 * H], F8, **ei)
    d["dwn"] = nc.dram_tensor("dwn", [128, TK, 4 * H], F8, **ei)
    d["dbias"] = nc.dram_tensor("dbias", [128, 2, 4 * H], F8, **ei)
    d["owt"] = nc.dram_tensor("owt", [128, HK, T], F8, **ei)
    d["obias"] = nc.dram_tensor("obias", [128, 2, T], F8, **ei)
    d["ident0"] = nc.dram_tensor("ident0", [128, 2, 128], F8, **ei)
    d["ident1"] = nc.dram_tensor("ident1", [128, 2, 128], F8, **ei)
    d["ones"] = nc.dram_tensor("ones", [128, 2, R], F8, **ei)
    d["outbuf"] = nc.dram_tensor("outbuf", [NS, TK, 128, R], BF,
                                 kind="ExternalOutput")
    import os
    if os.environ.get("KDEBUG") == "1":
        d["dbg_emb"] = nc.dram_tensor("dbg_emb", [128, HK, R], F8,
                                      kind="ExternalOutput")
        d["dbg_ge"] = nc.dram_tensor("dbg_ge", [128, G, R], F8,
                                     kind="ExternalOutput")
        d["dbg_h1"] = nc.dram_tensor("dbg_h1", [128, HK, R], F8,
                                     kind="ExternalOutput")
        d["dbg_c1"] = nc.dram_tensor("dbg_c1", [128, HK, R], F32,
                                     kind="ExternalOutput")
        d["dbg_po"] = nc.dram_tensor("dbg_po", [128, TK, R], F32,
                                     kind="ExternalOutput")
    return d


def _mm(nc, out, w, x, start, stop):
    return nc.tensor.matmul(out, w, x, start=start, stop=stop, perf_mode=DR)


def _body(nc, tc, d):
    import contextlib
    with contextlib.ExitStack() as ctx:
        # ---------- persistent tiles ----------
        Pp = ctx.enter_context(tc.tile_pool(name="persist", bufs=1))
        t_ones = Pp.tile([128, 2, R], F8, tag="ones")
        nc.sync.dma_start(t_ones[:], d["ones"][:])
        t_id0 = Pp.tile([128, 2, 128], F8, tag="ident0")
        nc.sync.dma_start(t_id0[:], d["ident0"][:])
        t_id1 = Pp.tile([128, 2, 128], F8, tag="ident1")
        nc.sync.dma_start(t_id1[:], d["ident1"][:])
        t_emb = Pp.tile([128, HK, R], F8, tag="emb")
        t_ge = Pp.tile([128, G, R], F8, tag="ge")
        Pdec = ctx.enter_context(tc.tile_pool(name="dec", bufs=1))
        t_h = [Pdec.tile([128, HK, R], F8, tag=f"hT{i}", name=f"hT{i}")
               for i in (0, 1)]
        t_c = Pdec.tile([128, HK, R], F32, tag="c")
        t_note8 = Pdec.tile([128, TK, R], F8, tag="note8")
        t_noteb = Pdec.tile([128, TK, R], BF, tag="noteb")
        t_obias = Pdec.tile([128, 2, T], F8, tag="obias")

        # ---------- conductor + ge ----------
        with tc.tile_pool(name="cond", bufs=1) as Pc, \
             tc.tile_pool(name="gew", bufs=1) as Pg, \
             tc.tile_pool(name="ctmp", bufs=1) as Pt, \
             tc.tile_pool(name="cps", bufs=2, space="PSUM") as PSc, \
             tc.tile_pool(name="geps", bufs=2, space="PSUM") as PSg:
            # conductor-critical DMAs on sync queue, in need order
            t_cwih = Pc.tile([128, ZK, 4 * H], F8, tag="cwih")
            nc.sync.dma_start(t_cwih[:], d["cwih"][:])
            t_zT = Pc.tile([128, ZK, R], F8, tag="zT")
            nc.sync.dma_start(t_zT[:], d["zT"][:])
            t_cbias = Pc.tile([128, 2, 4 * H], F8, tag="cbias")
            nc.sync.dma_start(t_cbias[:], d["cbias"][:])
            t_cwhh = Pc.tile([128, HK, 4 * H], F8, tag="cwhh")
            nc.sync.dma_start(t_cwhh[:], d["cwhh"][:])
            # remaining inputs, same sync queue, in order of first use
            # (all DMA transfers serialize through the shared DMA engines)
            t_dwe = Pg.tile([128, HK, 4 * H], F8, tag="dwe")
            nc.sync.dma_start(t_dwe[:], d["dwe"][:])
            t_dbias = Pg.tile([128, 2, 4 * H], F8, tag="dbias")
            nc.sync.dma_start(t_dbias[:], d["dbias"][:])
            t_dwhh = Pdec.tile([128, HK, 4 * H], F8, tag="dwhh")
            nc.sync.dma_start(t_dwhh[:], d["dwhh"][:])
            nc.sync.dma_start(t_h[0][:], d["h0T"][:])
            nc.sync.dma_start(t_c[:], d["c0T"][:])
            t_owt = Pdec.tile([128, HK, T], F8, tag="owt")
            nc.sync.dma_start(t_owt[:], d["owt"][:])
            nc.sync.dma_start(t_obias[:], d["obias"][:])
            t_dwn = Pdec.tile([128, TK, 4 * H], F8, tag="dwn")
            nc.sync.dma_start(t_dwn[:], d["dwn"][:])

            t_cc = Pc.tile([128, HK, Bc], F32, tag="cc")

            def emit_ge(lv):
                cs = slice(lv * Bc, (lv + 1) * Bc)
                pg = PSg.tile([128, G, Bc], F32, tag="gps", name="pg")
                for m in range(G):
                    ms = slice(m * 128, (m + 1) * 128)
                    out = pg[:, m, :]
                    _mm(nc, out, t_dbias[:, :, ms], t_ones[:, :, :Bc],
                        True, False)
                    for k in (0, 2, 4, 6):
                        _mm(nc, out, t_dwe[:, k:k + 2, ms],
                            t_emb[:, k:k + 2, cs], False, (k == 6))
                nc.vector.tensor_copy(t_ge[:, :, cs], pg[:])

            # gate order in psum dim1: 0=i, 1=f, 2=o, 3=g
            QMAP = (0, 1, 3, 2)  # psum q -> pytorch gate index
            GLAG = 4  # delay ge so the dwe DMA hides under early levels
            for lv in range(L):
                cs = slice(lv * Bc, (lv + 1) * Bc)
                ps = PSc.tile([128, 4, HK, Bc], F32, tag="cps")
                for q in range(4):
                    for p in range(HK):
                        m = QMAP[q] * HK + p
                        ms = slice(m * 128, (m + 1) * 128)
                        out = ps[:, q, p, :]
                        _mm(nc, out, t_cbias[:, :, ms], t_ones[:, :, :Bc],
                            True, False)
                        for k in (0, 2):
                            last = (lv == 0 and k == 2)
                            _mm(nc, out, t_cwih[:, k:k + 2, ms],
                                t_zT[:, k:k + 2, cs], False, last)
                        if lv > 0:
                            prev = slice((lv - 1) * Bc, lv * Bc)
                            for k in (0, 2, 4, 6):
                                _mm(nc, out, t_cwhh[:, k:k + 2, ms],
                                    t_emb[:, k:k + 2, prev], False, (k == 6))
                tifo = Pt.tile([128, 4, HK, Bc], BF, tag="ifo")
                nc.scalar.activation(tifo[:], ps[:], AF.Sigmoid, scale=INV)
                # g-rows of weights are pre-doubled: tanh(g) = 2*sig(2g) - 1
                # (in-place on the g slab to save SBUF)
                nc.vector.tensor_scalar(tifo[:, 3], tifo[:, 3], 2.0, -1.0,
                                        mybir.AluOpType.mult,
                                        mybir.AluOpType.add)
                tm1 = Pt.tile([128, HK, Bc], BF, tag="tm1")
                nc.vector.tensor_mul(tm1[:], tifo[:, 0], tifo[:, 3])
                if lv == 0:
                    nc.vector.tensor_copy(t_cc[:], tm1[:])
                else:
                    tm2 = Pt.tile([128, HK, Bc], F32, tag="tm2")
                    nc.vector.tensor_mul(tm2[:], tifo[:, 1], t_cc[:])
                    nc.vector.tensor_add(t_cc[:], tm1[:], tm2[:])
                tcn = Pt.tile([128, HK, Bc], BF, tag="tcn")
                nc.scalar.activation(tcn[:], t_cc[:], AF.Tanh)
                nc.vector.tensor_mul(t_emb[:, :, cs], tifo[:, 2], tcn[:])
                if lv >= GLAG:
                    emit_ge(lv - GLAG)
            for lv in range(L - GLAG, L):
                emit_ge(lv)

        import os
        if os.environ.get("KDEBUG") == "1":
            nc.sync.dma_start(d["dbg_emb"][:], t_emb[:])
            nc.sync.dma_start(d["dbg_ge"][:], t_ge[:])

        # ---------- decoder: 16 note steps over 512 rows ----------
        with tc.tile_pool(name="dtmp", bufs=3) as Pd, \
             tc.tile_pool(name="dps", bufs=3, space="PSUM") as PSd, \
             tc.tile_pool(name="dpso", bufs=1, space="PSUM") as PSo:
            for t in range(NS):
                hin = t_h[t % 2]
                hout = t_h[(t + 1) % 2]
                po = [None, None]
                for p in range(HK):
                    # psA: (i, f); psB: (o, g)
                    psA = PSd.tile([128, 2, R], F32, tag="dps", name="psA")
                    psB = PSd.tile([128, 2, R], F32, tag="dps", name="psB")
                    for pst, sl, gate in ((psA, 0, 0), (psA, 1, 1),
                                          (psB, 0, 3), (psB, 1, 2)):
                        m = gate * HK + p
                        ms = slice(m * 128, (m + 1) * 128)
                        out = pst[:, sl, :]
                        if m < G - 1:
                            _mm(nc, out, t_id0[:], t_ge[:, m:m + 2, :],
                                True, False)
                        else:
                            _mm(nc, out, t_id1[:], t_ge[:, m - 1:m + 1, :],
                                True, False)
                        for k in (0, 2, 4, 6):
                            last = (t == 0 and k == 6)
                            _mm(nc, out, t_dwhh[:, k:k + 2, ms],
                                hin[:, k:k + 2, :], False, last)
                        if t > 0:
                            for k in (0, 2):
                                _mm(nc, out, t_dwn[:, k:k + 2, ms],
                                    t_note8[:, k:k + 2, :], False, (k == 2))
                    tif = Pd.tile([128, 2, R], BF, tag="tif")
                    tob = Pd.tile([128, 2, R], BF, tag="tob")
                    nc.scalar.activation(tif[:], psA[:], AF.Sigmoid,
                                         scale=INV)
                    nc.scalar.activation(tob[:], psB[:], AF.Sigmoid,
                                         scale=INV)
                    tgg = Pd.tile([128, R], BF, tag="tgg")
                    nc.vector.tensor_scalar(tgg[:], tob[:, 1], 2.0, -1.0,
                                            mybir.AluOpType.mult,
                                            mybir.AluOpType.add)
                    tm1 = Pd.tile([128, R], BF, tag="tm1")
                    tm2 = Pd.tile([128, R], F32, tag="tm2")
                    nc.vector.tensor_mul(tm1[:], tif[:, 0], tgg[:])
                    nc.vector.tensor_mul(tm2[:], tif[:, 1], t_c[:, p, :])
                    nc.vector.tensor_add(t_c[:, p, :], tm1[:], tm2[:])
                    tcn = Pd.tile([128, R], BF, tag="tcn")
                    nc.scalar.activation(tcn[:], t_c[:, p, :], AF.Tanh)
                    heng = nc.gpsimd if p < 6 else nc.vector
                    heng.tensor_mul(hout[:, p, :], tob[:, 0], tcn[:])
                    if p == 6:
                        # oproj round 1 partial: h chunks 0..5 written by now
                        # (program order); final pair emitted after p==7.
                        po[0] = PSo.tile([128, 2, R], F32, tag="dpo",
                                         name="po0")
                        for tk in (0, 1):
                            ts_ = slice(tk * 128, (tk + 1) * 128)
                            out = po[0][:, tk, :]
                            _mm(nc, out, t_obias[:, :, ts_], t_ones[:],
                                True, False)
                            for k in (0, 2, 4):
                                _mm(nc, out, t_owt[:, k:k + 2, ts_],
                                    hout[:, k:k + 2, :], False, False)
                    if p == 7:
                        for tk in (0, 1):
                            ts_ = slice(tk * 128, (tk + 1) * 128)
                            _mm(nc, po[0][:, tk, :], t_owt[:, 6:8, ts_],
                                hout[:, 6:8, :], False, True)
                # oproj: drain round r fully before filling round r+1
                # (PSo bufs=1: po[r+1] reuses po[r]'s banks)
                for r in (0, 1):
                    if r == 1:
                        po[1] = PSo.tile([128, 2, R], F32, tag="dpo",
                                         name="po1")
                        for tk in (2, 3):
                            ts_ = slice(tk * 128, (tk + 1) * 128)
                            out = po[1][:, tk - 2, :]
                            _mm(nc, out, t_obias[:, :, ts_], t_ones[:],
                                True, False)
                            for k in (0, 2, 4, 6):
                                _mm(nc, out, t_owt[:, k:k + 2, ts_],
                                    hout[:, k:k + 2, :], False, (k == 6))
                    nb = t_noteb[:, 2 * r:2 * r + 2, :]
                    nc.scalar.activation(nb, po[r][:], AF.Sigmoid, scale=INV)
                    if t < NS - 1:
                        eng = nc.vector if r == 0 else nc.gpsimd
                        eng.tensor_copy(t_note8[:, 2 * r:2 * r + 2, :], nb)
                    for tk in (2 * r, 2 * r + 1):
                        nc.sync.dma_start(d["outbuf"][t, tk],
                                          t_noteb[:, tk, :])
                if t == 0 and "dbg_h1" in d:
                    nc.sync.dma_start(d["dbg_h1"][:], hout[:])
                    nc.sync.dma_start(d["dbg_c1"][:], t_c[:])
                    tpo = Pd.tile([128, TK, R], F32, tag="tpo")
                    nc.vector.tensor_copy(tpo[:, 0:2, :], po[0][:])
                    nc.vector.tensor_copy(tpo[:, 2:4, :], po[1][:])
                    nc.sync.dma_start(d["dbg_po"][:], tpo[:])


_CACHE = {}


def _build():
    if "nc" not in _CACHE:
        nc = bacc.Bacc("TRN2", target_bir_lowering=False, debug=False,
                       num_devices=NCORES)
        d = _declare(nc)
        with tile.TileContext(nc) as tc:
            _body(nc, tc, d)
        nc.compile()
        _CACHE["nc"] = nc
    return _CACHE["nc"]


def _q8(x):
    return np.asarray(x, np.float32).astype(f8)


def _hi_lo_rows(v):
    """[J] f32 -> hi/lo fp8 rows approximating 16*v."""
    hi = _q8(SCALE * v)
    lo = _q8(SCALE * np.asarray(v, np.float32) - hi.astype(np.float32))
    return hi, lo


def _feat_major(W):
    """[J, K] -> [128, K/128, J] (stationary lhsT chunk layout), f32."""
    J, K = W.shape
    return np.ascontiguousarray(
        np.asarray(W, np.float32).reshape(J, K // 128, 128).transpose(2, 1, 0))


def _pack_inputs(inputs):
    z = np.asarray(inputs["z"], np.float32)
    dec_h0 = np.asarray(inputs["dec_h0"], np.float32)
    dec_c0 = np.asarray(inputs["dec_c0"], np.float32)

    def dg(W):
        """Double the g-gate rows [2H:3H] (tanh(g) = 2*sig(2g) - 1 trick)."""
        W = np.array(W, np.float32)
        W[2 * H:3 * H] *= 2.0
        return W

    cond_b = dg(np.asarray(inputs["cond_bih"] + inputs["cond_bhh"],
                           np.float32))
    dec_b = dg(np.asarray(inputs["dec_bih"] + inputs["dec_bhh"], np.float32))
    out_b = np.asarray(inputs["out_b"], np.float32)
    cond_Wih = dg(inputs["cond_Wih"])
    cond_Whh = dg(inputs["cond_Whh"])
    dec_Wih = dg(np.asarray(inputs["dec_Wih"], np.float32))
    dec_Whh = dg(inputs["dec_Whh"])

    def bias_pair(v, n):
        t = np.zeros((128, 2, n), f8)
        t[0, 0], t[0, 1] = _hi_lo_rows(v)
        return t

    ident0 = np.zeros((128, 2, 128), f8)
    ident0[:, 0][np.arange(128), np.arange(128)] = np.float32(1.0)
    ident1 = np.zeros((128, 2, 128), f8)
    ident1[:, 1][np.arange(128), np.arange(128)] = np.float32(1.0)
    ones = np.zeros((128, 2, R), f8)
    ones[0] = np.float32(1.0)

    shared = {
        "ones": ones,
        "ident0": ident0,
        "ident1": ident1,
        "cbias": bias_pair(cond_b, 4 * H),
        "dbias": bias_pair(dec_b, 4 * H),
        "obias": bias_pair(out_b, T),
        "cwih": _q8(SCALE * _feat_major(cond_Wih)),
        "cwhh": _q8(SCALE * _feat_major(cond_Whh)),
        "dwe": _q8(SCALE * _feat_major(dec_Wih[:, :H])),
        "dwn": _q8(SCALE * _feat_major(dec_Wih[:, H:])),
        "dwhh": _q8(SCALE * _feat_major(dec_Whh)),
        "owt": _q8(SCALE * _feat_major(inputs["out_W"])),
    }

    z_lv = z[:, np.arange(L) * L, 0, :]           # [B, L, Z]
    in_maps = []
    for c in range(NCORES):
        bs = slice(c * Bc, (c + 1) * Bc)
        zc = z_lv[bs]                              # [Bc, L, Z]
        zT = np.ascontiguousarray(
            zc.reshape(Bc, L, ZK, 128).transpose(3, 2, 1, 0).reshape(
                128, ZK, R))
        h0 = dec_h0[:, bs, :]                      # [L, Bc, H]
        h0T = np.ascontiguousarray(
            h0.reshape(L, Bc, HK, 128).transpose(3, 2, 0, 1).reshape(
                128, HK, R))
        c0 = dec_c0[:, bs, :]
        c0T = np.ascontiguousarray(
            c0.reshape(L, Bc, HK, 128).transpose(3, 2, 0, 1).reshape(
                128, HK, R))
        m = dict(shared)
        m["zT"] = _q8(zT)
        m["h0T"] = _q8(h0T)
        m["c0T"] = c0T.astype(np.float32)
        in_maps.append(m)
    return in_maps


def _unpack_outputs(core_outs):
    notes = np.empty((B, L * NS, T), np.float32)
    for c, arr in enumerate(core_outs):
        a = arr.astype(np.float32).reshape(
            NS, TK, 128, L, Bc).transpose(4, 3, 0, 1, 2)
        notes[c * Bc:(c + 1) * Bc] = a.reshape(Bc, L, NS, T).reshape(
            Bc, L * NS, T)
    return notes


def kernel(**inputs):
    nc = _build()
    in_maps = _pack_inputs(inputs)
    res = run_bass_kernel_spmd(nc, in_maps, list(range(NCORES)))
    return _unpack_outputs([r["outbuf"] for r in res.results])



# revision 14
# speedup vs baseline: 1.0482x; 1.0482x over previous
"""Trainium2 Bass kernel for nn_Decoder (MusicVAE-style hierarchical decoder).

fp8 DoubleRow version. Strategy (8 NeuronCores, data-parallel over batch):
  - All matmuls in fp8 e4m3 with DoubleRow perf mode (2 k-tiles of 128 per
    instruction), weights pre-scaled by 16 (descale folded into the
    activation `scale=1/16`), PSUM accumulation in fp32.
  - Biases are injected into PSUM via DoubleRow matmuls whose moving operand
    is nonzero only on partition 0 (hi/lo fp8 pair for near-exact bias).
  - Conductor (16 sequential levels, batch 32/core): gates recomputed from
    z each level (no gz precompute); per-level ge table (= dec_Wih[:, :H]
    @ emb + dec_b, scaled by 16) is computed during the conductor and
    stored as an fp8 hi+lo pair.
  - Decoder: 16 levels batched (512 rows/core), 16 note steps. Gate psum =
    ge-inject (identity stationary, hi/lo pair) + Whh·h (4 DR) + Wn·note
    (2 DR). Output projection interleaved so the note->next-gates latency
    hides under gate matmuls of the next step.
  - h and note kept in fp8; c state fp32; activations bf16 out.
"""
import numpy as np
import ml_dtypes

import concourse.bacc as bacc
import concourse.tile as tile
import concourse.mybir as mybir
from concourse.bass_utils import run_bass_kernel_spmd

f8 = ml_dtypes.float8_e4m3
bf16 = ml_dtypes.bfloat16
F32 = mybir.dt.float32
BF = mybir.dt.bfloat16
F8 = mybir.dt.float8e4
AF = mybir.ActivationFunctionType
DR = mybir.MatmulPerfMode.DoubleRow

NCORES = 8
B, Z, H, T = 256, 512, 1024, 512
L, NS = 16, 16
Bc = B // NCORES            # 32 batch rows per core
R = L * Bc                  # 512 decoder rows per core
HK, TK, ZK = H // 128, T // 128, Z // 128   # 8, 4, 4
G = 4 * H // 128            # 32 gate chunks of 128
SCALE = 16.0
INV = 1.0 / SCALE


def _declare(nc):
    d = {}
    ei = dict(kind="ExternalInput")
    d["zT"] = nc.dram_tensor("zT", [128, ZK, R], F8, **ei)
    d["h0T"] = nc.dram_tensor("h0T", [128, HK, R], F8, **ei)
    d["c0T"] = nc.dram_tensor("c0T", [128, HK, R], F32, **ei)
    d["cwih"] = nc.dram_tensor("cwih", [128, ZK, 4 * H], F8, **ei)
    d["cwhh"] = nc.dram_tensor("cwhh", [128, HK, 4 * H], F8, **ei)
    d["cbias"] = nc.dram_tensor("cbias", [128, 2, 4 * H], F8, **ei)
    d["dwe"] = nc.dram_tensor("dwe", [128, HK, 4 * H], F8, **ei)
    d["dwhh"] = nc.dram_tensor("dwhh", [128, HK, 4 * H], F8, **ei)
    d["dwn"] = nc.dram_tensor("dwn", [128, TK, 4 * H], F8, **ei)
    d["dbias"] = nc.dram_tensor("dbias", [128, 2, 4 * H], F8, **ei)
    d["owt"] = nc.dram_tensor("owt", [128, HK, T], F8, **ei)
    d["obias"] = nc.dram_tensor("obias", [128, 2, T], F8, **ei)
    d["ident0"] = nc.dram_tensor("ident0", [128, 2, 128], F8, **ei)
    d["ident1"] = nc.dram_tensor("ident1", [128, 2, 128], F8, **ei)
    d["ones"] = nc.dram_tensor("ones", [128, 2, R], F8, **ei)
    d["outbuf"] = nc.dram_tensor("outbuf", [NS, TK, 128, R], BF,
                                 kind="ExternalOutput")
    import os
    if os.environ.get("KDEBUG") == "1":
        d["dbg_emb"] = nc.dram_tensor("dbg_emb", [128, HK, R], F8,
                                      kind="ExternalOutput")
        d["dbg_ge"] = nc.dram_tensor("dbg_ge", [128, G, R], F8,
                                     kind="ExternalOutput")
        d["dbg_h1"] = nc.dram_tensor("dbg_h1", [128, HK, R], F8,
                                     kind="ExternalOutput")
        d["dbg_c1"] = nc.dram_tensor("dbg_c1", [128, HK, R], F32,
                                     kind="ExternalOutput")
        d["dbg_po"] = nc.dram_tensor("dbg_po", [128, TK, R], F32,
                                     kind="ExternalOutput")
    return d


def _mm(nc, out, w, x, start, stop):
    return nc.tensor.matmul(out, w, x, start=start, stop=stop, perf_mode=DR)


def _body(nc, tc, d):
    import contextlib
    with contextlib.ExitStack() as ctx:
        # ---------- persistent tiles ----------
        Pp = ctx.enter_context(tc.tile_pool(name="persist", bufs=1))
        t_ones = Pp.tile([128, 2, R], F8, tag="ones")
        nc.sync.dma_start(t_ones[:], d["ones"][:])
        t_id0 = Pp.tile([128, 2, 128], F8, tag="ident0")
        nc.sync.dma_start(t_id0[:], d["ident0"][:])
        t_id1 = Pp.tile([128, 2, 128], F8, tag="ident1")
        nc.sync.dma_start(t_id1[:], d["ident1"][:])
        t_emb = Pp.tile([128, HK, R], F8, tag="emb")
        t_ge = Pp.tile([128, G, R], F8, tag="ge")
        Pdec = ctx.enter_context(tc.tile_pool(name="dec", bufs=1))
        t_h = [Pdec.tile([128, HK, R], F8, tag=f"hT{i}", name=f"hT{i}")
               for i in (0, 1)]
        t_c = Pdec.tile([128, HK, R], F32, tag="c")
        t_note8 = Pdec.tile([128, TK, R], F8, tag="note8")
        t_noteb = Pdec.tile([128, TK, R], BF, tag="noteb")
        t_obias = Pdec.tile([128, 2, T], F8, tag="obias")

        # ---------- conductor + ge ----------
        with tc.tile_pool(name="cond", bufs=1) as Pc, \
             tc.tile_pool(name="gew", bufs=1) as Pg, \
             tc.tile_pool(name="ctmp", bufs=1) as Pt, \
             tc.tile_pool(name="cps", bufs=2, space="PSUM") as PSc, \
             tc.tile_pool(name="geps", bufs=2, space="PSUM") as PSg:
            # conductor-critical DMAs on sync queue, in need order
            t_cwih = Pc.tile([128, ZK, 4 * H], F8, tag="cwih")
            nc.sync.dma_start(t_cwih[:], d["cwih"][:])
            t_zT = Pc.tile([128, ZK, R], F8, tag="zT")
            nc.sync.dma_start(t_zT[:], d["zT"][:])
            t_cbias = Pc.tile([128, 2, 4 * H], F8, tag="cbias")
            nc.sync.dma_start(t_cbias[:], d["cbias"][:])
            t_cwhh = Pc.tile([128, HK, 4 * H], F8, tag="cwhh")
            nc.sync.dma_start(t_cwhh[:], d["cwhh"][:])
            # remaining inputs, same sync queue, in order of first use
            # (all DMA transfers serialize through the shared DMA engines)
            t_dwe = Pg.tile([128, HK, 4 * H], F8, tag="dwe")
            nc.sync.dma_start(t_dwe[:], d["dwe"][:])
            t_dbias = Pg.tile([128, 2, 4 * H], F8, tag="dbias")
            nc.sync.dma_start(t_dbias[:], d["dbias"][:])
            t_dwhh = Pdec.tile([128, HK, 4 * H], F8, tag="dwhh")
            nc.sync.dma_start(t_dwhh[:], d["dwhh"][:])
            nc.sync.dma_start(t_h[0][:], d["h0T"][:])
            nc.sync.dma_start(t_c[:], d["c0T"][:])
            t_owt = Pdec.tile([128, HK, T], F8, tag="owt")
            nc.sync.dma_start(t_owt[:], d["owt"][:])
            nc.sync.dma_start(t_obias[:], d["obias"][:])
            t_dwn = Pdec.tile([128, TK, 4 * H], F8, tag="dwn")
            nc.sync.dma_start(t_dwn[:], d["dwn"][:])

            t_cc = Pc.tile([128, HK, Bc], F32, tag="cc")

            def emit_ge(lv):
                cs = slice(lv * Bc, (lv + 1) * Bc)
                pg = PSg.tile([128, G, Bc], F32, tag="gps", name="pg")
                for m in range(G):
                    ms = slice(m * 128, (m + 1) * 128)
                    out = pg[:, m, :]
                    _mm(nc, out, t_dbias[:, :, ms], t_ones[:, :, :Bc],
                        True, False)
                    for k in (0, 2, 4, 6):
                        _mm(nc, out, t_dwe[:, k:k + 2, ms],
                            t_emb[:, k:k + 2, cs], False, (k == 6))
                nc.vector.tensor_copy(t_ge[:, :, cs], pg[:])

            # gate order in psum dim1: 0=i, 1=f, 2=o, 3=g
            QMAP = (0, 1, 3, 2)  # psum q -> pytorch gate index
            GLAG = 4  # delay ge so the dwe DMA hides under early levels
            for lv in range(L):
                cs = slice(lv * Bc, (lv + 1) * Bc)
                ps = PSc.tile([128, 4, HK, Bc], F32, tag="cps")
                for q in range(4):
                    for p in range(HK):
                        m = QMAP[q] * HK + p
                        ms = slice(m * 128, (m + 1) * 128)
                        out = ps[:, q, p, :]
                        _mm(nc, out, t_cbias[:, :, ms], t_ones[:, :, :Bc],
                            True, False)
                        for k in (0, 2):
                            last = (lv == 0 and k == 2)
                            _mm(nc, out, t_cwih[:, k:k + 2, ms],
                                t_zT[:, k:k + 2, cs], False, last)
                        if lv > 0:
                            prev = slice((lv - 1) * Bc, lv * Bc)
                            for k in (0, 2, 4, 6):
                                _mm(nc, out, t_cwhh[:, k:k + 2, ms],
                                    t_emb[:, k:k + 2, prev], False, (k == 6))
                tifo = Pt.tile([128, 4, HK, Bc], BF, tag="ifo")
                nc.scalar.activation(tifo[:], ps[:], AF.Sigmoid, scale=INV)
                # g-rows of weights are pre-doubled: tanh(g) = 2*sig(2g) - 1
                # (in-place on the g slab to save SBUF)
                nc.vector.tensor_scalar(tifo[:, 3], tifo[:, 3], 2.0, -1.0,
                                        mybir.AluOpType.mult,
                                        mybir.AluOpType.add)
                tm1 = Pt.tile([128, HK, Bc], BF, tag="tm1")
                nc.vector.tensor_mul(tm1[:], tifo[:, 0], tifo[:, 3])
                if lv == 0:
                    nc.vector.tensor_copy(t_cc[:], tm1[:])
                else:
                    tm2 = Pt.tile([128, HK, Bc], F32, tag="tm2")
                    nc.vector.tensor_mul(tm2[:], tifo[:, 1], t_cc[:])
                    nc.vector.tensor_add(t_cc[:], tm1[:], tm2[:])
                tcn = Pt.tile([128, HK, Bc], BF, tag="tcn")
                nc.scalar.activation(tcn[:], t_cc[:], AF.Tanh)
                nc.vector.tensor_mul(t_emb[:, :, cs], tifo[:, 2], tcn[:])
                if lv >= GLAG:
                    emit_ge(lv - GLAG)
            for lv in range(L - GLAG, L):
                emit_ge(lv)

        import os
        if os.environ.get("KDEBUG") == "1":
            nc.sync.dma_start(d["dbg_emb"][:], t_emb[:])
            nc.sync.dma_start(d["dbg_ge"][:], t_ge[:])

        # ---------- decoder: 16 note steps over 512 rows ----------
        with tc.tile_pool(name="dtmp", bufs=3) as Pd, \
             tc.tile_pool(name="dps", bufs=4, space="PSUM") as PSd:
            for t in range(NS):
                hin = t_h[t % 2]
                hout = t_h[(t + 1) % 2]
                po = [None, None]

                def finish_chunk(pp, ptob):
                    """tanh(c) + h-mul for chunk pp (deferred one iteration
                    so the Act queue never blocks the next chunk's gates)."""
                    tcn = Pd.tile([128, R], BF, tag="tcn", name="tcn")
                    nc.scalar.activation(tcn[:], t_c[:, pp, :], AF.Tanh)
                    heng = nc.gpsimd if pp < 6 else nc.vector
                    heng.tensor_mul(hout[:, pp, :], ptob[:, 0], tcn[:])

                prev = None
                for p in range(HK):
                    # psA: (i, f); psB: (o, g)
                    psA = PSd.tile([128, 2, R], F32, tag="dps", name="psA")
                    psB = PSd.tile([128, 2, R], F32, tag="dps", name="psB")
                    for pst, sl, gate in ((psA, 0, 0), (psA, 1, 1),
                                          (psB, 0, 3), (psB, 1, 2)):
                        m = gate * HK + p
                        ms = slice(m * 128, (m + 1) * 128)
                        out = pst[:, sl, :]
                        if m < G - 1:
                            _mm(nc, out, t_id0[:], t_ge[:, m:m + 2, :],
                                True, False)
                        else:
                            _mm(nc, out, t_id1[:], t_ge[:, m - 1:m + 1, :],
                                True, False)
                        for k in (0, 2, 4, 6):
                            last = (t == 0 and k == 6)
                            _mm(nc, out, t_dwhh[:, k:k + 2, ms],
                                hin[:, k:k + 2, :], False, last)
                        if t > 0:
                            for k in (0, 2):
                                _mm(nc, out, t_dwn[:, k:k + 2, ms],
                                    t_note8[:, k:k + 2, :], False, (k == 2))
                    tif = Pd.tile([128, 2, R], BF, tag="tif")
                    tob = Pd.tile([128, 2, R], BF, tag="tob", name="tob")
                    nc.scalar.activation(tif[:], psA[:], AF.Sigmoid,
                                         scale=INV)
                    nc.scalar.activation(tob[:], psB[:], AF.Sigmoid,
                                         scale=INV)
                    if prev is not None:
                        finish_chunk(*prev)
                        if prev[0] == 5:
                            # oproj round 1 partial (h chunks 0..5 ready)
                            po[0] = PSd.tile([128, 2, R], F32, tag="dps",
                                             name="po0")
                            for tk in (0, 1):
                                ts_ = slice(tk * 128, (tk + 1) * 128)
                                out = po[0][:, tk, :]
                                _mm(nc, out, t_obias[:, :, ts_], t_ones[:],
                                    True, False)
                                for k in (0, 2, 4):
                                    _mm(nc, out, t_owt[:, k:k + 2, ts_],
                                        hout[:, k:k + 2, :], False, False)
                    tgg = Pd.tile([128, R], BF, tag="tgg")
                    nc.vector.tensor_scalar(tgg[:], tob[:, 1], 2.0, -1.0,
                                            mybir.AluOpType.mult,
                                            mybir.AluOpType.add)
                    tm1 = Pd.tile([128, R], BF, tag="tm1")
                    tm2 = Pd.tile([128, R], F32, tag="tm2")
                    nc.vector.tensor_mul(tm1[:], tif[:, 0], tgg[:])
                    nc.vector.tensor_mul(tm2[:], tif[:, 1], t_c[:, p, :])
                    nc.vector.tensor_add(t_c[:, p, :], tm1[:], tm2[:])
                    prev = (p, tob)
                finish_chunk(*prev)
                for tk in (0, 1):
                    ts_ = slice(tk * 128, (tk + 1) * 128)
                    _mm(nc, po[0][:, tk, :], t_owt[:, 6:8, ts_],
                        hout[:, 6:8, :], False, True)
                # oproj round 2 + sigmoids + fp8 casts + output DMA
                for r in (0, 1):
                    if r == 1:
                        po[1] = PSd.tile([128, 2, R], F32, tag="dps",
                                         name="po1")
                        for tk in (2, 3):
                            ts_ = slice(tk * 128, (tk + 1) * 128)
                            out = po[1][:, tk - 2, :]
                            _mm(nc, out, t_obias[:, :, ts_], t_ones[:],
                                True, False)
                            for k in (0, 2, 4, 6):
                                _mm(nc, out, t_owt[:, k:k + 2, ts_],
                                    hout[:, k:k + 2, :], False, (k == 6))
                    nb = t_noteb[:, 2 * r:2 * r + 2, :]
                    nc.scalar.activation(nb, po[r][:], AF.Sigmoid, scale=INV)
                    if t < NS - 1:
                        eng = nc.vector if r == 0 else nc.gpsimd
                        eng.tensor_copy(t_note8[:, 2 * r:2 * r + 2, :], nb)
                    for tk in (2 * r, 2 * r + 1):
                        nc.sync.dma_start(d["outbuf"][t, tk],
                                          t_noteb[:, tk, :])
                if t == 0 and "dbg_h1" in d:
                    nc.sync.dma_start(d["dbg_h1"][:], hout[:])
                    nc.sync.dma_start(d["dbg_c1"][:], t_c[:])
                    tpo = Pd.tile([128, TK, R], F32, tag="tpo")
                    nc.vector.tensor_copy(tpo[:, 0:2, :], po[0][:])
                    nc.vector.tensor_copy(tpo[:, 2:4, :], po[1][:])
                    nc.sync.dma_start(d["dbg_po"][:], tpo[:])


_CACHE = {}


def _build():
    if "nc" not in _CACHE:
        nc = bacc.Bacc("TRN2", target_bir_lowering=False, debug=False,
                       num_devices=NCORES)
        d = _declare(nc)
        with tile.TileContext(nc) as tc:
            _body(nc, tc, d)
        nc.compile()
        _CACHE["nc"] = nc
    return _CACHE["nc"]


def _q8(x):
    return np.asarray(x, np.float32).astype(f8)


def _hi_lo_rows(v):
    """[J] f32 -> hi/lo fp8 rows approximating 16*v."""
    hi = _q8(SCALE * v)
    lo = _q8(SCALE * np.asarray(v, np.float32) - hi.astype(np.float32))
    return hi, lo


def _feat_major(W):
    """[J, K] -> [128, K/128, J] (stationary lhsT chunk layout), f32."""
    J, K = W.shape
    return np.ascontiguousarray(
        np.asarray(W, np.float32).reshape(J, K // 128, 128).transpose(2, 1, 0))


def _pack_inputs(inputs):
    z = np.asarray(inputs["z"], np.float32)
    dec_h0 = np.asarray(inputs["dec_h0"], np.float32)
    dec_c0 = np.asarray(inputs["dec_c0"], np.float32)

    def dg(W):
        """Double the g-gate rows [2H:3H] (tanh(g) = 2*sig(2g) - 1 trick)."""
        W = np.array(W, np.float32)
        W[2 * H:3 * H] *= 2.0
        return W

    cond_b = dg(np.asarray(inputs["cond_bih"] + inputs["cond_bhh"],
                           np.float32))
    dec_b = dg(np.asarray(inputs["dec_bih"] + inputs["dec_bhh"], np.float32))
    out_b = np.asarray(inputs["out_b"], np.float32)
    cond_Wih = dg(inputs["cond_Wih"])
    cond_Whh = dg(inputs["cond_Whh"])
    dec_Wih = dg(np.asarray(inputs["dec_Wih"], np.float32))
    dec_Whh = dg(inputs["dec_Whh"])

    def bias_pair(v, n):
        t = np.zeros((128, 2, n), f8)
        t[0, 0], t[0, 1] = _hi_lo_rows(v)
        return t

    ident0 = np.zeros((128, 2, 128), f8)
    ident0[:, 0][np.arange(128), np.arange(128)] = np.float32(1.0)
    ident1 = np.zeros((128, 2, 128), f8)
    ident1[:, 1][np.arange(128), np.arange(128)] = np.float32(1.0)
    ones = np.zeros((128, 2, R), f8)
    ones[0] = np.float32(1.0)

    shared = {
        "ones": ones,
        "ident0": ident0,
        "ident1": ident1,
        "cbias": bias_pair(cond_b, 4 * H),
        "dbias": bias_pair(dec_b, 4 * H),
        "obias": bias_pair(out_b, T),
        "cwih": _q8(SCALE * _feat_major(cond_Wih)),
        "cwhh": _q8(SCALE * _feat_major(cond_Whh)),
        "dwe": _q8(SCALE * _feat_major(dec_Wih[:, :H])),
        "dwn": _q8(SCALE * _feat_major(dec_Wih[:, H:])),
        "dwhh": _q8(SCALE * _feat_major(dec_Whh)),
        "owt": _q8(SCALE * _feat_major(inputs["out_W"])),
    }

    z_lv = z[:, np.arange(L) * L, 0, :]           # [B, L, Z]
    in_maps = []
    for c in range(NCORES):
        bs = slice(c * Bc, (c + 1) * Bc)
        zc = z_lv[bs]                              # [Bc, L, Z]
        zT = np.ascontiguousarray(
            zc.reshape(Bc, L, ZK, 128).transpose(3, 2, 1, 0).reshape(
                128, ZK, R))
        h0 = dec_h0[:, bs, :]                      # [L, Bc, H]
        h0T = np.ascontiguousarray(
            h0.reshape(L, Bc, HK, 128).transpose(3, 2, 0, 1).reshape(
                128, HK, R))
        c0 = dec_c0[:, bs, :]
        c0T = np.ascontiguousarray(
            c0.reshape(L, Bc, HK, 128).transpose(3, 2, 0, 1).reshape(
                128, HK, R))
        m = dict(shared)
        m["zT"] = _q8(zT)
        m["h0T"] = _q8(h0T)
        m["c0T"] = c0T.astype(np.float32)
        in_maps.append(m)
    return in_maps


def _unpack_outputs(core_outs):
    notes = np.empty((B, L * NS, T), np.float32)
    for c, arr in enumerate(core_outs):
        a = arr.astype(np.float32).reshape(
            NS, TK, 128, L, Bc).transpose(4, 3, 0, 1, 2)
        notes[c * Bc:(c + 1) * Bc] = a.reshape(Bc, L, NS, T).reshape(
            Bc, L * NS, T)
    return notes


def kernel(**inputs):
    nc = _build()
    in_maps = _pack_inputs(inputs)
    res = run_bass_kernel_spmd(nc, in_maps, list(range(NCORES)))
    return _unpack_outputs([r["outbuf"] for r in res.results])

